# revision 6
# baseline (speedup 1.0000x reference)
"""Overlapping-chunk extraction kernel for Trainium2 (Bass).

Computes out[b, j, c, f] = x[b, 125*j + c, f] for j in [0, 255), c in [0, 250),
i.e. 255 half-overlapping chunks of length 250 from a (16, 32000, 64) signal.

Strategy (pure data movement, memory-bound). Shard batch across 8 cores
(2 samples per core); per sample:

  1. Inbound: ONE contiguous 8.19 MB HBM->SBUF DMA into a haloed layout
     buf[p, 0:16000] = x[16000p : 16000(p+1)]  (64 KB descriptors).
     Each input byte is read from HBM exactly once (the direct HBM->HBM
     variant reads the overlapping source twice).
  2. Halo fill: SBUF->SBUF DMA buf[p, 16000:24000] <- buf[p+1, 0:8000].
     On-chip only; sb2sb is immune to the small-descriptor HBM penalty.
  3. Outbound: 2 DMAs, both built from 64000-byte descriptors (measured
     ~3x faster per byte than 32 KB descriptors for HBM writes):
       evens: chunk 2p   = buf[p, 0:16000]     -> y stride-32000 blocks
       odds:  chunk 2k+1 = buf[k, 8000:24000]  -> y stride-32000 blocks
     (the halo makes odd chunks contiguous within one partition).

All DMAs on GPSIMD (SWDGE): HWDGE dynamic DMA costs ~1.1 us per
descriptor row and is 5x slower for these multi-row patterns.

Per-core HBM traffic: 16.4 MB read + 32.6 MB written = 49.0 MB
(vs 65.3 MB for direct HBM->HBM).
"""

import numpy as np

import concourse.bass as bass
import concourse.mybir as mybir
from concourse.bass_utils import run_bass_kernel_spmd

# Problem shape (hardcoded per contract)
B, T, F = 16, 32000, 64
N_CORES = 8
S = B // N_CORES          # samples per core = 2
NFC = 128                 # non-overlapping chunks per sample
CHUNK = 250               # frames per chunk
NOV = 2 * NFC - 1         # 255 overlapped output chunks
PART_FREE = CHUNK * F     # 16000 fp32 per chunk = 64000 B (1 descriptor)
HALF_FREE = PART_FREE // 2  # 8000 fp32 = 125 frames (chunk advance)
HALO_FREE = PART_FREE + HALF_FREE  # 24000 fp32 per partition incl. halo
SAMPLE_IN = T * F         # 2_048_000 fp32 per input sample
SAMPLE_OUT = NOV * PART_FREE  # 4_080_000 fp32 per output sample

_NC_CACHE = {}


def build_module(repeat=1, name="chunkop", schedule="phased"):
    """Build the kernel program; `repeat` chains the whole kernel R times
    back-to-back (semaphore-gated) for HW timing via differencing."""
    nc = bass.Bass(trn_type="TRN2", name=name)
    x = nc.dram_tensor("x", [S, T, F], mybir.dt.float32, kind="ExternalInput")
    y = nc.dram_tensor(
        "y", [S, NOV, CHUNK, F], mybir.dt.float32, kind="ExternalOutput"
    )
    x_t = x[:, :, :].tensor
    y_t = y[:, :, :, :].tensor

    with (
        nc.sbuf_tensor([NFC, HALO_FREE], mybir.dt.float32) as buf0,
        nc.sbuf_tensor([NFC, HALO_FREE], mybir.dt.float32) as buf1,
        nc.semaphore("s_in0") as s_in0,
        nc.semaphore("s_in1") as s_in1,
        nc.semaphore("s_halo0") as s_halo0,
        nc.semaphore("s_halo1") as s_halo1,
        nc.semaphore("s_done0") as s_done0,
        nc.semaphore("s_done1") as s_done1,
        nc.Block() as block,
    ):
        bufs = [buf0, buf1]
        s_in = [s_in0, s_in1]
        s_halo = [s_halo0, s_halo1]
        s_done = [s_done0, s_done1]

        @block.gpsimd
        def _(g):
            with nc.allow_non_contiguous_dma(reason="strided chunk writes"):
                def load(s, r):
                    if r > 0:
                        # evens+odds of rep r-1 must release buffer s
                        g.wait_ge(s_done[s], 32 * r)
                    src = bass.AP(
                        x_t, s * SAMPLE_IN, [[PART_FREE, NFC], [1, PART_FREE]]
                    )
                    g.dma_start(bufs[s][:, 0:PART_FREE], src).then_inc(
                        s_in[s], 16
                    )

                def store_head(s, r):
                    """After in(s) lands: halo fill + even-chunk writes."""
                    buf = bufs[s]
                    g.wait_ge(s_in[s], 16 * (r + 1))
                    # halo: buf[k, 16000:24000) <- buf[k+1, 0:8000)
                    g.dma_start(
                        buf[0 : NFC - 1, PART_FREE:HALO_FREE],
                        buf[1:NFC, 0:HALF_FREE],
                    ).then_inc(s_halo[s], 16)
                    # even chunks j=2p: y[32000p : 32000p+16000)
                    dst = bass.AP(
                        y_t,
                        s * SAMPLE_OUT,
                        [[2 * PART_FREE, NFC], [1, PART_FREE]],
                    )
                    g.dma_start(dst, buf[:, 0:PART_FREE]).then_inc(
                        s_done[s], 16
                    )

                def store_tail(s, r):
                    """After the halo lands: odd-chunk writes."""
                    buf = bufs[s]
                    g.wait_ge(s_halo[s], 16 * (r + 1))
                    # odd chunks j=2k+1: y[32000k+16000 : 32000k+32000)
                    #   = buf[k, 8000:24000) (contiguous thanks to halo)
                    dst = bass.AP(
                        y_t,
                        s * SAMPLE_OUT + PART_FREE,
                        [[2 * PART_FREE, NFC - 1], [1, PART_FREE]],
                    )
                    g.dma_start(
                        dst, buf[0 : NFC - 1, HALF_FREE:HALO_FREE]
                    ).then_inc(s_done[s], 16)

                if schedule == "pipelined":
                    for r in range(repeat):
                        # in1 queues behind sample 0's stores so the read
                        # stream overlaps the write stream
                        load(0, r)
                        store_head(0, r)
                        load(1, r)
                        store_tail(0, r)
                        store_head(1, r)
                        store_tail(1, r)
                else:  # "phased": all reads, then all writes per rep —
                    # avoids HBM read/write bus-turnaround mixing cost
                    for r in range(repeat):
                        load(0, r)
                        load(1, r)
                        store_head(0, r)
                        store_head(1, r)
                        store_tail(0, r)
                        store_tail(1, r)
                g.wait_ge(s_done[0], 32 * repeat)
                g.wait_ge(s_done[1], 32 * repeat)

    return nc


def get_module():
    if "nc" not in _NC_CACHE:
        _NC_CACHE["nc"] = build_module()
    return _NC_CACHE["nc"]


def kernel(x):
    x = np.ascontiguousarray(np.asarray(x), dtype=np.float32)
    assert x.shape == (B, T, F), x.shape
    nc = get_module()
    in_maps = [{"x": x[i * S : (i + 1) * S]} for i in range(N_CORES)]
    res = run_bass_kernel_spmd(nc, in_maps, core_ids=list(range(N_CORES)))
    return np.concatenate([r["y"] for r in res.results], axis=0)


# revision 8
# speedup vs baseline: 3.7752x; 3.7752x over previous
"""Overlapping-chunk extraction kernel for Trainium2 (Bass).

Computes out[b, j, c, f] = x[b, 125*j + c, f] for j in [0, 255), c in [0, 250),
i.e. 255 half-overlapping chunks of length 250 from a (16, 32000, 64) signal.
Batch is sharded across 8 cores (2 samples per core).

Key HW finding (measured): an SWDGE DMA runs at ~300-340 GB/s only when its
SBUF side is a FULL flat 128-partition region ([[W,128],[1,W]], offset 0);
any offset/stride/partition-subset falls into a slow generic descriptor path
(52-230 GB/s). HWDGE dynamic DMA costs ~1.1 us per descriptor row (5x slower
here). So the kernel uses only full-flat SBUF-side SWDGE DMAs:

  per sample s:
    in:    x[s] (contig 8.19 MB) -> bufE_s [128 x 16000]   (1x HBM read)
    evens: bufE_s -> y even chunks (128 blocks @ stride 32000)
    odds:  bufO   -> y odd chunks: 128 blocks @ stride 32000, offset 16000.
           Only 127 odd chunks exist; the 128th "dummy" block lands in
           sample s+1's chunk-0 region (rewritten later by its evens DMA)
           or, for the last sample, in a 64 KB pad appended to y.

  bufO (odd chunks, one chunk per partition) is assembled ON-CHIP so HBM
  is only read once:
    - DVE copies the same-partition half: bufO[k, 0:8000] = bufE[k, 8000:16000]
    - TensorE shifts partitions via matmul with a subdiagonal 0/1 matrix:
      bufO[k, 8000:16000] = bufE[k+1, 0:8000]  (through PSUM, DVE evacuates)
      fp32 matmul by exact 1.0/0.0 weights -> error ~2^-17, well under tol.

Per-core HBM traffic: 16.4 MB read + 32.8 MB written at ~300 GB/s.
"""

import numpy as np

import concourse.bass as bass
import concourse.mybir as mybir
from concourse.bass_utils import run_bass_kernel_spmd

# Problem shape (hardcoded per contract)
B, T, F = 16, 32000, 64
N_CORES = 8
S = B // N_CORES          # samples per core = 2
NFC = 128                 # non-overlapping chunks per sample
CHUNK = 250               # frames per chunk
NOV = 2 * NFC - 1         # 255 overlapped output chunks
PART_FREE = CHUNK * F     # 16000 fp32 per chunk = 64000 B
HALF_FREE = PART_FREE // 2  # 8000 fp32 = 125 frames (chunk advance)
SAMPLE_IN = T * F         # 2_048_000 fp32 per input sample
SAMPLE_OUT = NOV * PART_FREE  # 4_080_000 fp32 per output sample
Y_PAD = S * SAMPLE_OUT + PART_FREE  # +64 KB so the last dummy block is in-bounds
MM_TILE = 500             # fp32 cols per matmul (one 2 KB PSUM bank)
N_MM = HALF_FREE // MM_TILE  # 16 matmuls per sample

_NC_CACHE = {}
F32 = mybir.dt.float32


def build_module(repeat=1, name="chunkop"):
    """Build the kernel program; `repeat` chains the whole kernel R times
    back-to-back (semaphore-gated) for HW timing via differencing."""
    nc = bass.Bass(trn_type="TRN2", name=name)
    x = nc.dram_tensor("x", [S, T, F], F32, kind="ExternalInput")
    y = nc.dram_tensor("y", [Y_PAD], F32, kind="ExternalOutput")
    x_t = x[:, :, :].tensor
    y_t = y[:].tensor

    with (
        nc.sbuf_tensor([NFC, PART_FREE], F32) as bufE0,
        nc.sbuf_tensor([NFC, PART_FREE], F32) as bufE1,
        nc.sbuf_tensor([NFC, PART_FREE], F32) as bufO,
        nc.sbuf_tensor([NFC, NFC - 1], F32) as sh,
        nc.psum_tensor([NFC, MM_TILE], F32) as ps0,
        nc.psum_tensor([NFC, MM_TILE], F32) as ps1,
        nc.semaphore("s_in0") as s_in0,
        nc.semaphore("s_in1") as s_in1,
        nc.semaphore("s_ev0") as s_ev0,
        nc.semaphore("s_ev1") as s_ev1,
        nc.semaphore("s_od0") as s_od0,
        nc.semaphore("s_od1") as s_od1,
        nc.semaphore("s_mm") as s_mm,
        nc.semaphore("s_evac") as s_evac,
        nc.semaphore("s_cp") as s_cp,
        nc.semaphore("s_init") as s_init,
        nc.Block() as block,
    ):
        bufE = [bufE0, bufE1]
        s_in = [s_in0, s_in1]
        s_ev = [s_ev0, s_ev1]
        s_od = [s_od0, s_od1]
        ps = [ps0, ps1]

        @block.gpsimd
        def _(g):
            with nc.allow_non_contiguous_dma(reason="strided chunk writes"):
                # one-time: subdiagonal shift matrix sh[p, m] = (p == m+1).
                # (bufO partition 127 stays uninitialized: that dummy odd
                # chunk only lands in overwritten or padded y regions.)
                g.memset(sh[:, :], 1.0)
                g.affine_select(
                    sh[:, :],
                    sh[:, :],
                    pattern=[[-1, NFC - 1]],
                    compare_op=mybir.AluOpType.is_equal,
                    fill=0.0,
                    base=-1,
                    channel_multiplier=1,
                ).then_inc(s_init, 1)

                for r in range(repeat):
                    for s_ in range(S):
                        k = 2 * r + s_
                        if r > 0:
                            # bufE_s readers of rep r-1 must finish:
                            # its evens DMA, PE matmuls, DVE half-copy
                            g.wait_ge(s_ev[s_], 16 * r)
                            g.wait_ge(s_mm, N_MM * (k - 1))
                            g.wait_ge(s_cp, k - 1)
                        src = bass.AP(
                            x_t,
                            s_ * SAMPLE_IN,
                            [[PART_FREE, NFC], [1, PART_FREE]],
                        )
                        g.dma_start(bufE[s_][:, :], src).then_inc(s_in[s_], 16)

                    # evens0: bufE0 -> y0 even chunks
                    g.wait_ge(s_in0, 16 * (r + 1))
                    dst = bass.AP(
                        y_t, 0, [[2 * PART_FREE, NFC], [1, PART_FREE]]
                    )
                    g.dma_start(dst, bufE0[:, :]).then_inc(s_ev0, 16)

                    # odds0: bufO -> y0 odd chunks (+ dummy into y1 chunk 0)
                    g.wait_ge(s_cp, 2 * r + 1)
                    g.wait_ge(s_evac, N_MM * (2 * r + 1))
                    dst = bass.AP(
                        y_t,
                        PART_FREE,
                        [[2 * PART_FREE, NFC], [1, PART_FREE]],
                    )
                    g.dma_start(dst, bufO[:, :]).then_inc(s_od0, 16)

                    # evens1: bufE1 -> y1 even chunks; must follow odds0's
                    # dummy write into y1 chunk 0
                    g.wait_ge(s_in1, 16 * (r + 1))
                    g.wait_ge(s_od0, 16 * (r + 1))
                    dst = bass.AP(
                        y_t,
                        SAMPLE_OUT,
                        [[2 * PART_FREE, NFC], [1, PART_FREE]],
                    )
                    g.dma_start(dst, bufE1[:, :]).then_inc(s_ev1, 16)

                    # odds1: bufO -> y1 odd chunks (+ dummy into pad)
                    g.wait_ge(s_cp, 2 * r + 2)
                    g.wait_ge(s_evac, N_MM * (2 * r + 2))
                    dst = bass.AP(
                        y_t,
                        SAMPLE_OUT + PART_FREE,
                        [[2 * PART_FREE, NFC], [1, PART_FREE]],
                    )
                    g.dma_start(dst, bufO[:, :]).then_inc(s_od1, 16)

                g.wait_ge(s_ev0, 16 * repeat)
                g.wait_ge(s_ev1, 16 * repeat)
                g.wait_ge(s_od0, 16 * repeat)
                g.wait_ge(s_od1, 16 * repeat)

        @block.tensor
        def _(t):
            t.wait_ge(s_init, 1)
            for k in range(2 * repeat):
                s_ = k % 2
                t.wait_ge(s_in[s_], 16 * (k // 2 + 1))
                for i in range(N_MM):
                    j = N_MM * k + i
                    if j >= 2:
                        # psum bank j%2 reused from matmul j-2, freed once
                        # DVE evacuated it (evac count >= j-1)
                        t.wait_ge(s_evac, j - 1)
                    t.matmul(
                        ps[j % 2][0 : NFC - 1, :],
                        sh[:, :],
                        bufE[s_][:, i * MM_TILE : (i + 1) * MM_TILE],
                        start=True,
                        stop=True,
                    ).then_inc(s_mm, 1)

        @block.vector
        def _(v):
            for k in range(2 * repeat):
                s_ = k % 2
                if k > 0:
                    # bufO reused: previous sample's odds DMA must finish
                    v.wait_ge(s_od[(k - 1) % 2], 16 * ((k - 1) // 2 + 1))
                v.wait_ge(s_in[s_], 16 * (k // 2 + 1))
                # same-partition half of each odd chunk
                v.tensor_copy(
                    bufO[0 : NFC - 1, 0:HALF_FREE],
                    bufE[s_][0 : NFC - 1, HALF_FREE:PART_FREE],
                ).then_inc(s_cp, 1)
                # shifted half: evacuate PE results PSUM -> bufO
                for i in range(N_MM):
                    j = N_MM * k + i
                    v.wait_ge(s_mm, j + 1)
                    v.tensor_copy(
                        bufO[
                            0 : NFC - 1,
                            HALF_FREE + i * MM_TILE : HALF_FREE + (i + 1) * MM_TILE,
                        ],
                        ps[j % 2][0 : NFC - 1, :],
                    ).then_inc(s_evac, 1)

    return nc


def get_module():
    if "nc" not in _NC_CACHE:
        _NC_CACHE["nc"] = build_module()
    return _NC_CACHE["nc"]


def kernel(x):
    x = np.ascontiguousarray(np.asarray(x), dtype=np.float32)
    assert x.shape == (B, T, F), x.shape
    nc = get_module()
    in_maps = [{"x": x[i * S : (i + 1) * S]} for i in range(N_CORES)]
    res = run_bass_kernel_spmd(nc, in_maps, core_ids=list(range(N_CORES)))
    outs = [
        r["y"][: S * SAMPLE_OUT].reshape(S, NOV, CHUNK, F) for r in res.results
    ]
    return np.concatenate(outs, axis=0)


# revision 25
# speedup vs baseline: 4.5297x; 1.1999x over previous
"""Overlapping-chunk extraction kernel for Trainium2 (Bass).

Computes out[b, j, c, f] = x[b, 125*j + c, f] for j in [0, 255), c in [0, 250),
i.e. 255 half-overlapping chunks of length 250 from a (16, 32000, 64) signal.
Batch is sharded across 8 cores (2 samples per core).

Key HW finding (measured): an SWDGE DMA runs at ~300-340 GB/s only when its
SBUF side is a FULL flat 128-partition region ([[W,128],[1,W]], offset 0);
any offset/stride/partition-subset falls into a slow generic descriptor path
(52-230 GB/s). HWDGE dynamic DMA costs ~1.1 us per descriptor row (5x slower
here). So the kernel uses only full-flat SBUF-side SWDGE DMAs:

  per sample s:
    in:    x[s] (contig 8.19 MB) -> bufE_s [128 x 16000]   (1x HBM read)
    evens: bufE_s -> y even chunks (128 blocks @ stride 32000)
    odds:  bufO   -> y odd chunks: 128 blocks @ stride 32000, offset 16000.
           Only 127 odd chunks exist; the 128th "dummy" block lands in
           sample s+1's chunk-0 region (rewritten later by its evens DMA)
           or, for the last sample, in a 64 KB pad appended to y.

  bufO (odd chunks, one chunk per partition) is assembled ON-CHIP so HBM
  is only read once:
    - DVE copies the same-partition half: bufO[k, 0:8000] = bufE[k, 8000:16000]
    - TensorE shifts partitions via matmul with a subdiagonal 0/1 matrix:
      bufO[k, 8000:16000] = bufE[k+1, 0:8000]  (through PSUM, DVE evacuates)
      fp32 matmul by exact 1.0/0.0 weights -> error ~2^-17, well under tol.

Per-core HBM traffic: 16.4 MB read + 32.8 MB written at ~300 GB/s.
"""

import numpy as np

import concourse.bass as bass
import concourse.mybir as mybir
from concourse.bass_utils import run_bass_kernel_spmd

# Problem shape (hardcoded per contract)
B, T, F = 16, 32000, 64
N_CORES = 8
S = B // N_CORES          # samples per core = 2
NFC = 128                 # non-overlapping chunks per sample
CHUNK = 250               # frames per chunk
NOV = 2 * NFC - 1         # 255 overlapped output chunks
PART_FREE = CHUNK * F     # 16000 fp32 per chunk = 64000 B
HALF_FREE = PART_FREE // 2  # 8000 fp32 = 125 frames (chunk advance)
SAMPLE_IN = T * F         # 2_048_000 fp32 per input sample
SAMPLE_OUT = NOV * PART_FREE  # 4_080_000 fp32 per output sample
Y_PAD = S * SAMPLE_OUT + PART_FREE  # +64 KB so the last dummy block is in-bounds
MM_TILE = 500             # fp32 cols per matmul (fits one 2 KB PSUM bank)
BANK_FP32 = 512           # PSUM bank stride in fp32
N_MM = HALF_FREE // MM_TILE  # 16 matmuls per sample, in 2 groups of 8 banks

_NC_CACHE = {}
F32 = mybir.dt.float32


def build_module(repeat=1, name="chunkop"):
    """Build the kernel program; `repeat` chains the whole kernel R times
    back-to-back (semaphore-gated) for HW timing via differencing."""
    nc = bass.Bass(trn_type="TRN2", name=name)
    x = nc.dram_tensor("x", [S, T, F], F32, kind="ExternalInput")
    y = nc.dram_tensor("y", [Y_PAD], F32, kind="ExternalOutput")
    x_t = x[:, :, :].tensor
    y_t = y[:].tensor

    from contextlib import ExitStack

    with ExitStack() as ctx:
        bufE = [
            ctx.enter_context(nc.sbuf_tensor(f"bufE{i}", [NFC, PART_FREE], F32))
            for i in range(2)
        ]
        bufO = ctx.enter_context(nc.sbuf_tensor("bufO", [NFC, PART_FREE], F32))
        sh = ctx.enter_context(nc.sbuf_tensor("sh", [NFC, NFC - 1], F32))
        ps = [
            ctx.enter_context(nc.psum_tensor(f"ps{i}", [NFC, MM_TILE], F32))
            for i in range(2)
        ]
        s_in = [ctx.enter_context(nc.semaphore(f"s_in{i}")) for i in range(2)]
        s_ev = [ctx.enter_context(nc.semaphore(f"s_ev{i}")) for i in range(2)]
        s_od = [ctx.enter_context(nc.semaphore(f"s_od{i}")) for i in range(2)]
        s_mm = ctx.enter_context(nc.semaphore("s_mm"))
        s_evac = ctx.enter_context(nc.semaphore("s_evac"))
        s_cp = ctx.enter_context(nc.semaphore("s_cp"))
        s_init = ctx.enter_context(nc.semaphore("s_init"))
        block = ctx.enter_context(nc.Block())
        s_in0, s_in1 = s_in
        s_ev0, s_ev1 = s_ev
        s_od0, s_od1 = s_od
        bufE0, bufE1 = bufE

        @block.gpsimd
        def _(g):
            with nc.allow_non_contiguous_dma(reason="strided chunk writes"):
                # one-time: subdiagonal shift matrix sh[p, m] = (p == m+1).
                # (bufO partition 127 stays uninitialized: that dummy odd
                # chunk only lands in overwritten or padded y regions.)
                g.memset(sh[:, :], 1.0)
                g.affine_select(
                    sh[:, :],
                    sh[:, :],
                    pattern=[[-1, NFC - 1]],
                    compare_op=mybir.AluOpType.is_equal,
                    fill=0.0,
                    base=-1,
                    channel_multiplier=1,
                ).then_inc(s_init, 1)

                for r in range(repeat):
                    for s_ in range(S):
                        k = 2 * r + s_
                        if r > 0:
                            # bufE_s readers of rep r-1 must finish:
                            # its evens DMA, PE matmuls, DVE half-copy
                            g.wait_ge(s_ev[s_], 16 * r)
                            g.wait_ge(s_mm, N_MM * (k - 1))
                            g.wait_ge(s_cp, k - 1)
                        src = bass.AP(
                            x_t,
                            s_ * SAMPLE_IN,
                            [[PART_FREE, NFC], [1, PART_FREE]],
                        )
                        g.dma_start(bufE[s_][:, :], src).then_inc(s_in[s_], 16)

                    # evens0: bufE0 -> y0 even chunks
                    g.wait_ge(s_in0, 16 * (r + 1))
                    dst = bass.AP(
                        y_t, 0, [[2 * PART_FREE, NFC], [1, PART_FREE]]
                    )
                    g.dma_start(dst, bufE0[:, :]).then_inc(s_ev0, 16)

                    # odds0: bufO -> y0 odd chunks (+ dummy into y1 chunk 0)
                    g.wait_ge(s_cp, 2 * r + 1)
                    g.wait_ge(s_evac, N_MM * (2 * r + 1))
                    dst = bass.AP(
                        y_t,
                        PART_FREE,
                        [[2 * PART_FREE, NFC], [1, PART_FREE]],
                    )
                    g.dma_start(dst, bufO[:, :]).then_inc(s_od0, 16)

                    # evens1: bufE1 -> y1 even chunks; must follow odds0's
                    # dummy write into y1 chunk 0
                    g.wait_ge(s_in1, 16 * (r + 1))
                    g.wait_ge(s_od0, 16 * (r + 1))
                    dst = bass.AP(
                        y_t,
                        SAMPLE_OUT,
                        [[2 * PART_FREE, NFC], [1, PART_FREE]],
                    )
                    g.dma_start(dst, bufE1[:, :]).then_inc(s_ev1, 16)

                    # odds1: bufO -> y1 odd chunks (+ dummy into pad)
                    g.wait_ge(s_cp, 2 * r + 2)
                    g.wait_ge(s_evac, N_MM * (2 * r + 2))
                    dst = bass.AP(
                        y_t,
                        SAMPLE_OUT + PART_FREE,
                        [[2 * PART_FREE, NFC], [1, PART_FREE]],
                    )
                    g.dma_start(dst, bufO[:, :]).then_inc(s_od1, 16)

                g.wait_ge(s_ev0, 16 * repeat)
                g.wait_ge(s_ev1, 16 * repeat)
                g.wait_ge(s_od0, 16 * repeat)
                g.wait_ge(s_od1, 16 * repeat)

        @block.tensor
        def _(t):
            t.wait_ge(s_init, 1)
            for k in range(2 * repeat):
                s_ = k % 2
                t.wait_ge(s_in[s_], 16 * (k // 2 + 1))
                for i in range(N_MM):
                    j = N_MM * k + i
                    if j >= 2:
                        # psum bank j%2 reused from matmul j-2, freed once
                        # DVE evacuated it (evac count >= j-1)
                        t.wait_ge(s_evac, j - 1)
                    t.matmul(
                        ps[j % 2][0 : NFC - 1, :],
                        sh[:, :],
                        bufE[s_][:, i * MM_TILE : (i + 1) * MM_TILE],
                        start=True,
                        stop=True,
                    ).then_inc(s_mm, 1)

        @block.vector
        def _(v):
            for k in range(2 * repeat):
                s_ = k % 2
                if k > 0:
                    # bufO reused: previous sample's odds DMA must finish
                    v.wait_ge(s_od[(k - 1) % 2], 16 * ((k - 1) // 2 + 1))
                v.wait_ge(s_in[s_], 16 * (k // 2 + 1))
                # same-partition half of each odd chunk
                v.tensor_copy(
                    bufO[0 : NFC - 1, 0:HALF_FREE],
                    bufE[s_][0 : NFC - 1, HALF_FREE:PART_FREE],
                ).then_inc(s_cp, 1)
                # shifted half: evacuate PE results PSUM -> bufO
                for i in range(N_MM):
                    j = N_MM * k + i
                    v.wait_ge(s_mm, j + 1)
                    v.tensor_copy(
                        bufO[
                            0 : NFC - 1,
                            HALF_FREE + i * MM_TILE :
                            HALF_FREE + (i + 1) * MM_TILE,
                        ],
                        ps[j % 2][0 : NFC - 1, :],
                    ).then_inc(s_evac, 1)

    return nc


def get_module():
    if "nc" not in _NC_CACHE:
        _NC_CACHE["nc"] = build_module()
    return _NC_CACHE["nc"]


def kernel(x):
    x = np.ascontiguousarray(np.asarray(x), dtype=np.float32)
    assert x.shape == (B, T, F), x.shape
    nc = get_module()
    in_maps = [{"x": x[i * S : (i + 1) * S]} for i in range(N_CORES)]
    res = run_bass_kernel_spmd(nc, in_maps, core_ids=list(range(N_CORES)))
    outs = [
        r["y"][: S * SAMPLE_OUT].reshape(S, NOV, CHUNK, F) for r in res.results
    ]
    return np.concatenate(outs, axis=0)
